# revision 2
# baseline (speedup 1.0000x reference)
"""GroupNorm2dInteger on 8 trn2 NeuronCores.

Data-parallel over batch: 64 samples -> 8 per core. Per sample (C*H*W =
524288 elems viewed as (128, 4096)):
  stats:  DVE reduce_sum + ACT Square(accum_out) give per-partition
          sum / sumsq; one PE matmul with a ones column folds the 128
          partitions for all 8 samples at once.
  scalars: quantized mean, var = E[x^2] - 2*mu*E[x] + mu^2, floor via
          (t - 0.5) -> int32 cast (round-to-nearest == floor for
          non-integer t), then the full integer isqrt is ONE gather from
          a 32768-entry table built on the host from the isqrt_lut
          input (exact integer math). Gather indices are byte offsets.
  normalize: fused (x - mu)*16*invsqrt in one tensor_scalar with
          per-partition AP scalars (broadcast via a ones-row matmul),
          floor-cast to int32, &255 for the uint output, *1/16 for the
          float output.

Walrus in this container encodes at most ONE sync wait per instruction;
_split_waits rewrites any instruction with more into a chain of
single-wait drains.
"""

import numpy as np

N, C, H, W = 64, 128, 64, 64
NCORES = 8
S = N // NCORES          # samples per core
P = 128                  # partitions
F = (C * H * W) // P     # 4096 free elems per partition
CHW = C * H * W          # 524288

VAR_W, VAR_F = 16, 8
SQRT2_FIX = 46340        # floor(sqrt(2) * 2^15)
LUT_POW = 5

_cache = {}


def _build_s16_table(lut: np.ndarray) -> np.ndarray:
    """TAB[v] = 16 * quantized inv_sqrt for var_int v, exact int math."""
    v = np.arange(1 << (VAR_W - 1), dtype=np.int64)  # var_int in [0, 32767]
    lut = lut.astype(np.int64)
    msb = np.zeros_like(v)
    vv = v.copy()
    for k in range(1, VAR_W):
        msb = np.where((vv >> k) > 0, k, msb)
    x_red = v << ((VAR_W - 1) - msb)
    idx = (x_red >> (VAR_W - 1 - LUT_POW)) & ((1 << LUT_POW) - 1)
    lv = lut[idx]
    e = 3 * VAR_F - msb
    e2 = e // 2
    odd = e - 2 * e2
    lv = np.where(odd == 1, (lv * SQRT2_FIX) >> (VAR_W - 1), lv)
    sh = (VAR_W - 1) - e2
    res = np.where(sh >= 0, lv >> np.maximum(sh, 0), lv << np.maximum(-sh, 0))
    res = np.where(v == 0, (1 << VAR_W) - 1, res)
    res = np.clip(res, 0, (1 << VAR_W) - 1)
    # inv_sqrt = res/256 then integer_floor_quantizer(.,16,8):
    # floor(inv_sqrt*256) = res, clipped to [-32768, 32767]
    res = np.minimum(res, (1 << (VAR_W - 1)) - 1)
    return (16.0 * res.astype(np.float64) / (1 << VAR_F)).astype(np.float32)


def _split_waits(nc):
    import bass_rust
    import concourse.mybir as mybir

    ctr = 0
    for f in nc.m.functions:
        for bb in f.blocks:
            insts = bb.instructions
            out = []
            changed = False
            for inst in insts:
                si = inst.sync_info
                waits = list(si.on_wait) if si is not None else []
                if len(waits) > 1:
                    changed = True
                    for i in range(0, len(waits) - 1):
                        d = mybir.InstDrain(
                            name=f"I-waitfix-{ctr}", ins=[], outs=[]
                        )
                        ctr += 1
                        d.engine = inst.engine
                        d.sync_info = bass_rust.SyncInfo(
                            on_wait=[waits[i]], on_update=[]
                        )
                        nc.register_instruction(d, overwrite=True)
                        out.append(d)
                    si.on_wait = waits[-1:]
                out.append(inst)
            if changed:
                insts.clear()
                insts.extend(out)


def _build_program():
    if "nc" in _cache:
        return _cache["nc"]
    import concourse.bass as bass
    import concourse.mybir as mybir
    import concourse.tile as tile

    f32 = mybir.dt.float32
    i32 = mybir.dt.int32
    Op = mybir.AluOpType

    nc = bass.Bass("TRN2", target_bir_lowering=False, debug=False)
    x_d = nc.dram_tensor("x", [S, P, F], f32, kind="ExternalInput")
    tab_d = nc.dram_tensor("tab", [1, 1 << (VAR_W - 1)], f32, kind="ExternalInput")
    of_d = nc.dram_tensor("outf", [S, P, F], f32, kind="ExternalOutput")
    ou_d = nc.dram_tensor("outu", [S, P, F], i32, kind="ExternalOutput")

    with tile.TileContext(nc) as tc:
        with (
            tc.tile_pool(name="xp", bufs=1) as xp,
            tc.tile_pool(name="fp", bufs=2) as fp,
            tc.tile_pool(name="up", bufs=2) as up,
            tc.tile_pool(name="sm", bufs=1) as sm,
            tc.tile_pool(name="ps", bufs=2, space="PSUM") as ps,
        ):
            stats = sm.tile([P, 16], f32)      # cols 0..7 sum, 8..15 sumsq
            ones_col = sm.tile([P, 1], f32)
            nc.vector.memset(ones_col[:], 1.0)
            ones_row = sm.tile([1, P], f32)
            nc.vector.memset(ones_row[:], 1.0)

            # ---- load + per-partition stats, one tile per sample ----
            xts = []
            for s in range(S):
                xt = xp.tile([P, F], f32, tag=f"x{s}")
                nc.sync.dma_start(xt[:], x_d.ap()[s])
                xts.append(xt)
                nc.vector.tensor_reduce(
                    stats[:, s : s + 1], xt[:], mybir.AxisListType.X, Op.add
                )
                usc = up.tile([P, F], i32, tag="u")
                nc.scalar.activation(
                    usc[:].bitcast(f32), xt[:],
                    mybir.ActivationFunctionType.Square,
                    accum_out=stats[:, 8 + s : 9 + s],
                )

            # ---- fold partitions: (1,16) = [sum_s | sumsq_s] ----
            cs_p = ps.tile([1, 16], f32)
            nc.tensor.matmul(cs_p[:], ones_col[:], stats[:], start=True, stop=True)
            R = sm.tile([1, 16], f32)
            nc.vector.tensor_copy(R[:], cs_p[:])

            # ---- per-sample scalar pipeline on partition-0 rows ----
            inv_n = 1.0 / CHW
            m16h = sm.tile([1, S], f32)   # 16*mean - 0.5
            nc.vector.tensor_scalar(
                m16h[:], R[:, 0:S], 16.0 * inv_n, -0.5, Op.mult, Op.add
            )
            mu16i = sm.tile([1, S], i32)
            nc.vector.tensor_copy(mu16i[:], m16h[:])      # rint == floor here
            mu16c = sm.tile([1, S], f32)
            nc.vector.tensor_scalar(
                mu16c[:], mu16i[:], -128.0, 127.0, Op.max, Op.min
            )
            mu_q = sm.tile([1, S], f32)
            nc.vector.tensor_scalar(mu_q[:], mu16c[:], 0.0625, None, Op.mult)

            mean = sm.tile([1, S], f32)
            nc.vector.tensor_scalar(mean[:], R[:, 0:S], inv_n, None, Op.mult)
            ex2 = sm.tile([1, S], f32)
            nc.vector.tensor_scalar(ex2[:], R[:, S : 2 * S], inv_n, None, Op.mult)
            t1 = sm.tile([1, S], f32)
            nc.vector.scalar_tensor_tensor(
                t1[:], mu_q[:], -2.0, mean[:], Op.mult, Op.mult
            )
            t2 = sm.tile([1, S], f32)
            nc.vector.scalar_tensor_tensor(
                t2[:], mu_q[:], 1.0, mu_q[:], Op.mult, Op.mult
            )
            v1 = sm.tile([1, S], f32)
            nc.vector.tensor_add(v1[:], ex2[:], t1[:])
            var = sm.tile([1, S], f32)
            nc.vector.tensor_add(var[:], v1[:], t2[:])

            v256h = sm.tile([1, S], f32)
            nc.vector.tensor_scalar(
                v256h[:], var[:], 256.0, -0.5, Op.mult, Op.add
            )
            v256c = sm.tile([1, S], f32)
            nc.vector.tensor_scalar(
                v256c[:], v256h[:], -0.5, 32766.5, Op.max, Op.min
            )
            vi = sm.tile([1, S], i32)
            nc.vector.tensor_copy(vi[:], v256c[:])        # rint == floor
            vi4 = sm.tile([1, S], i32)                    # byte offsets
            nc.vector.tensor_scalar(vi4[:], vi[:], 4, None, Op.mult)

            s16 = sm.tile([1, S], f32)
            import concourse.bass as _b
            nc.gpsimd.indirect_dma_start(
                s16[:], None,
                tab_d.ap(), _b.IndirectOffsetOnAxis(ap=vi4[:], axis=1),
            )

            brow = sm.tile([1, 2 * S], f32)
            nc.vector.tensor_copy(brow[:, 0:S], mu_q[:])
            nc.vector.tensor_copy(brow[:, S : 2 * S], s16[:])
            bc_p = ps.tile([P, 2 * S], f32)
            nc.tensor.matmul(bc_p[:], ones_row[:], brow[:], start=True, stop=True)
            SC = sm.tile([P, 2 * S], f32)
            nc.vector.tensor_copy(SC[:], bc_p[:])

            # ---- normalize + quantize per sample ----
            for s in range(S):
                xt = xts[s]
                nc.vector.tensor_scalar(
                    xt[:], xt[:], SC[:, s : s + 1], SC[:, S + s : S + s + 1],
                    Op.subtract, Op.mult,
                )
                ut = up.tile([P, F], i32, tag="u")
                nc.vector.tensor_scalar(ut[:], xt[:], -0.5, None, Op.add)
                ft = fp.tile([P, F], f32, tag="f")
                nc.vector.tensor_scalar(ft[:], ut[:], 0.0625, None, Op.mult)
                nc.sync.dma_start(of_d.ap()[s], ft[:])
                nc.vector.tensor_scalar(ut[:], ut[:], 255, None, Op.bitwise_and)
                nc.sync.dma_start(ou_d.ap()[s], ut[:])

    _split_waits(nc)
    _cache["nc"] = nc
    return nc


def _run(x: np.ndarray, isqrt_lut: np.ndarray, trace: bool = False):
    from concourse import bass_utils

    nc = _build_program()
    tab = _build_s16_table(np.asarray(isqrt_lut)).reshape(1, -1)
    xr = np.ascontiguousarray(np.asarray(x, dtype=np.float32).reshape(N, P, F))
    in_maps = [
        {"x": np.ascontiguousarray(xr[c * S : (c + 1) * S]), "tab": tab}
        for c in range(NCORES)
    ]
    res = bass_utils.run_bass_kernel_spmd(
        nc, in_maps, list(range(NCORES)), trace=trace
    )
    outf = np.empty((N, P, F), dtype=np.float32)
    outu = np.empty((N, P, F), dtype=np.int32)
    for c in range(NCORES):
        outf[c * S : (c + 1) * S] = res.results[c]["outf"]
        outu[c * S : (c + 1) * S] = res.results[c]["outu"]
    return (
        outf.reshape(N, C, H, W),
        outu.reshape(N, C, H, W),
        res.exec_time_ns,
    )


def kernel(x: np.ndarray, isqrt_lut: np.ndarray):
    f, u, _ = _run(x, isqrt_lut, trace=False)
    return f, u


# revision 3
# speedup vs baseline: 1.0537x; 1.0537x over previous
"""GroupNorm2dInteger on 8 trn2 NeuronCores.

Data-parallel over batch: 64 samples -> 8 per core. Per sample (C*H*W =
524288 elems viewed as (128, 4096)):
  stats:  DVE reduce_sum + ACT Square(accum_out) give per-partition
          sum / sumsq; one PE matmul with a ones column folds the 128
          partitions for all 8 samples at once.
  scalars: quantized mean, var = E[x^2] - 2*mu*E[x] + mu^2, floor via
          (t - 0.5) -> int32 cast (round-to-nearest == floor for
          non-integer t), then the full integer isqrt is ONE gather from
          a 32768-entry table built on the host from the isqrt_lut
          input (exact integer math). Gather indices are byte offsets.
  normalize: fused (x - mu)*16*invsqrt in one tensor_scalar with
          per-partition AP scalars (broadcast via a ones-row matmul),
          floor-cast to int32, &255 for the uint output, *1/16 for the
          float output.

Walrus in this container encodes at most ONE sync wait per instruction;
_split_waits rewrites any instruction with more into a chain of
single-wait drains.
"""

import numpy as np

N, C, H, W = 64, 128, 64, 64
NCORES = 8
S = N // NCORES          # samples per core
P = 128                  # partitions
F = (C * H * W) // P     # 4096 free elems per partition
CHW = C * H * W          # 524288

VAR_W, VAR_F = 16, 8
SQRT2_FIX = 46340        # floor(sqrt(2) * 2^15)
LUT_POW = 5

_cache = {}


def _build_s16_table(lut: np.ndarray) -> np.ndarray:
    """TAB[v] = 16 * quantized inv_sqrt for var_int v, exact int math."""
    v = np.arange(1 << (VAR_W - 1), dtype=np.int64)  # var_int in [0, 32767]
    lut = lut.astype(np.int64)
    msb = np.zeros_like(v)
    vv = v.copy()
    for k in range(1, VAR_W):
        msb = np.where((vv >> k) > 0, k, msb)
    x_red = v << ((VAR_W - 1) - msb)
    idx = (x_red >> (VAR_W - 1 - LUT_POW)) & ((1 << LUT_POW) - 1)
    lv = lut[idx]
    e = 3 * VAR_F - msb
    e2 = e // 2
    odd = e - 2 * e2
    lv = np.where(odd == 1, (lv * SQRT2_FIX) >> (VAR_W - 1), lv)
    sh = (VAR_W - 1) - e2
    res = np.where(sh >= 0, lv >> np.maximum(sh, 0), lv << np.maximum(-sh, 0))
    res = np.where(v == 0, (1 << VAR_W) - 1, res)
    res = np.clip(res, 0, (1 << VAR_W) - 1)
    # inv_sqrt = res/256 then integer_floor_quantizer(.,16,8):
    # floor(inv_sqrt*256) = res, clipped to [-32768, 32767]
    res = np.minimum(res, (1 << (VAR_W - 1)) - 1)
    return (16.0 * res.astype(np.float64) / (1 << VAR_F)).astype(np.float32)


def _split_waits(nc):
    import bass_rust
    import concourse.mybir as mybir

    ctr = 0
    for f in nc.m.functions:
        for bb in f.blocks:
            insts = bb.instructions
            out = []
            changed = False
            for inst in insts:
                si = inst.sync_info
                waits = list(si.on_wait) if si is not None else []
                if len(waits) > 1:
                    changed = True
                    for i in range(0, len(waits) - 1):
                        d = mybir.InstDrain(
                            name=f"I-waitfix-{ctr}", ins=[], outs=[]
                        )
                        ctr += 1
                        d.engine = inst.engine
                        d.sync_info = bass_rust.SyncInfo(
                            on_wait=[waits[i]], on_update=[]
                        )
                        nc.register_instruction(d, overwrite=True)
                        out.append(d)
                    si.on_wait = waits[-1:]
                out.append(inst)
            if changed:
                insts.clear()
                insts.extend(out)


def _build_program():
    if "nc" in _cache:
        return _cache["nc"]
    import concourse.bass as bass
    import concourse.mybir as mybir
    import concourse.tile as tile

    f32 = mybir.dt.float32
    i32 = mybir.dt.int32
    Op = mybir.AluOpType

    nc = bass.Bass("TRN2", target_bir_lowering=False, debug=False)
    x_d = nc.dram_tensor("x", [S, P, F], f32, kind="ExternalInput")
    tab_d = nc.dram_tensor("tab", [1, 1 << (VAR_W - 1)], f32, kind="ExternalInput")
    of_d = nc.dram_tensor("outf", [S, P, F], f32, kind="ExternalOutput")
    ou_d = nc.dram_tensor("outu", [S, P, F], i32, kind="ExternalOutput")

    with tile.TileContext(nc) as tc:
        with (
            tc.tile_pool(name="xp", bufs=1) as xp,
            tc.tile_pool(name="fp", bufs=2) as fp,
            tc.tile_pool(name="up", bufs=2) as up,
            tc.tile_pool(name="sm", bufs=1) as sm,
            tc.tile_pool(name="ps", bufs=2, space="PSUM") as ps,
        ):
            stats = sm.tile([P, 16], f32)      # cols 0..7 sum, 8..15 sumsq
            ones_col = sm.tile([P, 1], f32)
            nc.vector.memset(ones_col[:], 1.0)
            ones_row = sm.tile([1, P], f32)
            nc.vector.memset(ones_row[:], 1.0)

            # ---- load + per-partition stats, one tile per sample ----
            # 3-stage reduction (16x16x16) keeps sequential-sum error ~48
            # ulp so the var*256 floor boundary never flips vs reference.
            xts = []
            for s in range(S):
                xt = xp.tile([P, F], f32, tag=f"x{s}")
                nc.sync.dma_start(xt[:], x_d.ap()[s])
                xts.append(xt)
                r1 = sm.tile([P, 256], f32, tag=f"r1{s % 2}")
                nc.vector.tensor_reduce(
                    r1[:].rearrange("p (a b) -> p a b", b=16),
                    xt[:].rearrange("p (a b c) -> p a b c", b=16, c=16),
                    mybir.AxisListType.X, Op.add,
                )
                r2 = sm.tile([P, 16], f32, tag=f"r2{s % 2}")
                nc.vector.tensor_reduce(
                    r2[:], r1[:].rearrange("p (a b) -> p a b", b=16),
                    mybir.AxisListType.X, Op.add,
                )
                nc.vector.tensor_reduce(
                    stats[:, s : s + 1], r2[:], mybir.AxisListType.X, Op.add
                )
                usc = up.tile([P, F], i32, tag="u")
                nc.scalar.activation(
                    usc[:].bitcast(f32), xt[:],
                    mybir.ActivationFunctionType.Square,
                )
                q1 = sm.tile([P, 256], f32, tag=f"q1{s % 2}")
                nc.vector.tensor_reduce(
                    q1[:].rearrange("p (a b) -> p a b", b=16),
                    usc[:].bitcast(f32).rearrange("p (a b c) -> p a b c", b=16, c=16),
                    mybir.AxisListType.X, Op.add,
                )
                q2 = sm.tile([P, 16], f32, tag=f"q2{s % 2}")
                nc.vector.tensor_reduce(
                    q2[:], q1[:].rearrange("p (a b) -> p a b", b=16),
                    mybir.AxisListType.X, Op.add,
                )
                nc.vector.tensor_reduce(
                    stats[:, 8 + s : 9 + s], q2[:], mybir.AxisListType.X, Op.add
                )

            # ---- fold partitions: (1,16) = [sum_s | sumsq_s] ----
            cs_p = ps.tile([1, 16], f32)
            nc.tensor.matmul(cs_p[:], ones_col[:], stats[:], start=True, stop=True)
            R = sm.tile([1, 16], f32)
            nc.vector.tensor_copy(R[:], cs_p[:])

            # ---- per-sample scalar pipeline on partition-0 rows ----
            inv_n = 1.0 / CHW
            m16h = sm.tile([1, S], f32)   # 16*mean - 0.5
            nc.vector.tensor_scalar(
                m16h[:], R[:, 0:S], 16.0 * inv_n, -0.5, Op.mult, Op.add
            )
            mu16i = sm.tile([1, S], i32)
            nc.vector.tensor_copy(mu16i[:], m16h[:])      # rint == floor here
            mu16c = sm.tile([1, S], f32)
            nc.vector.tensor_scalar(
                mu16c[:], mu16i[:], -128.0, 127.0, Op.max, Op.min
            )
            mu_q = sm.tile([1, S], f32)
            nc.vector.tensor_scalar(mu_q[:], mu16c[:], 0.0625, None, Op.mult)

            mean = sm.tile([1, S], f32)
            nc.vector.tensor_scalar(mean[:], R[:, 0:S], inv_n, None, Op.mult)
            ex2 = sm.tile([1, S], f32)
            nc.vector.tensor_scalar(ex2[:], R[:, S : 2 * S], inv_n, None, Op.mult)
            t1 = sm.tile([1, S], f32)
            nc.vector.scalar_tensor_tensor(
                t1[:], mu_q[:], -2.0, mean[:], Op.mult, Op.mult
            )
            t2 = sm.tile([1, S], f32)
            nc.vector.scalar_tensor_tensor(
                t2[:], mu_q[:], 1.0, mu_q[:], Op.mult, Op.mult
            )
            v1 = sm.tile([1, S], f32)
            nc.vector.tensor_add(v1[:], ex2[:], t1[:])
            var = sm.tile([1, S], f32)
            nc.vector.tensor_add(var[:], v1[:], t2[:])

            v256h = sm.tile([1, S], f32)
            nc.vector.tensor_scalar(
                v256h[:], var[:], 256.0, -0.5, Op.mult, Op.add
            )
            v256c = sm.tile([1, S], f32)
            nc.vector.tensor_scalar(
                v256c[:], v256h[:], -0.5, 32766.5, Op.max, Op.min
            )
            vi = sm.tile([1, S], i32)
            nc.vector.tensor_copy(vi[:], v256c[:])        # rint == floor
            vi4 = sm.tile([1, S], i32)                    # byte offsets
            nc.vector.tensor_scalar(vi4[:], vi[:], 4, None, Op.mult)

            s16 = sm.tile([1, S], f32)
            import concourse.bass as _b
            nc.gpsimd.indirect_dma_start(
                s16[:], None,
                tab_d.ap(), _b.IndirectOffsetOnAxis(ap=vi4[:], axis=1),
            )

            brow = sm.tile([1, 2 * S], f32)
            nc.vector.tensor_copy(brow[:, 0:S], mu_q[:])
            nc.vector.tensor_copy(brow[:, S : 2 * S], s16[:])
            bc_p = ps.tile([P, 2 * S], f32)
            nc.tensor.matmul(bc_p[:], ones_row[:], brow[:], start=True, stop=True)
            SC = sm.tile([P, 2 * S], f32)
            nc.vector.tensor_copy(SC[:], bc_p[:])

            # ---- normalize + quantize per sample ----
            for s in range(S):
                xt = xts[s]
                nc.vector.tensor_scalar(
                    xt[:], xt[:], SC[:, s : s + 1], SC[:, S + s : S + s + 1],
                    Op.subtract, Op.mult,
                )
                ut = up.tile([P, F], i32, tag="u")
                nc.vector.tensor_scalar(ut[:], xt[:], -0.5, None, Op.add)
                ft = fp.tile([P, F], f32, tag="f")
                nc.vector.tensor_scalar(ft[:], ut[:], 0.0625, None, Op.mult)
                nc.sync.dma_start(of_d.ap()[s], ft[:])
                nc.vector.tensor_scalar(ut[:], ut[:], 255, None, Op.bitwise_and)
                nc.sync.dma_start(ou_d.ap()[s], ut[:])

    _split_waits(nc)
    _cache["nc"] = nc
    return nc


def _run(x: np.ndarray, isqrt_lut: np.ndarray, trace: bool = False):
    from concourse import bass_utils

    nc = _build_program()
    tab = _build_s16_table(np.asarray(isqrt_lut)).reshape(1, -1)
    xr = np.ascontiguousarray(np.asarray(x, dtype=np.float32).reshape(N, P, F))
    in_maps = [
        {"x": np.ascontiguousarray(xr[c * S : (c + 1) * S]), "tab": tab}
        for c in range(NCORES)
    ]
    res = bass_utils.run_bass_kernel_spmd(
        nc, in_maps, list(range(NCORES)), trace=trace
    )
    outf = np.empty((N, P, F), dtype=np.float32)
    outu = np.empty((N, P, F), dtype=np.int32)
    for c in range(NCORES):
        outf[c * S : (c + 1) * S] = res.results[c]["outf"]
        outu[c * S : (c + 1) * S] = res.results[c]["outu"]
    return (
        outf.reshape(N, C, H, W),
        outu.reshape(N, C, H, W),
        res.exec_time_ns,
    )


def kernel(x: np.ndarray, isqrt_lut: np.ndarray):
    f, u, _ = _run(x, isqrt_lut, trace=False)
    return f, u
